# revision 5
# baseline (speedup 1.0000x reference)
"""Trainium2 Bass kernel for nn_DecoderWithoutAttention.

Strategy (memory-regime target):
  - The module = LSTM decoder + vocab projection. The vocab projection
    (h @ W_fc.T, V=32000) dominates both FLOPs (103 GFLOP) and memory
    traffic (the 401 MB [B,T,V] output). The LSTM recurrence is tiny
    (13 GFLOP, sequential) and runs on host to produce the masked hidden
    states HS[B*T, H]; zero rows for inactive (t,b) make their
    predictions exactly 0 (b_fc == 0 in this module).
  - Device: tensor-parallel shard of the projection over vocab across 8
    cores: PRED_c[3200pad, 4000] = HS[3200, 512] @ W_fcT[:, c*4000:...].
    float32r matmuls (full PE rate at free dim >= 256), fp32 PSUM
    accumulate over K=512, DMA out 50 MB/core.
"""

import numpy as np

import concourse.bass as bass
import concourse.mybir as mybir
import concourse.tile as tile
from concourse.bass_utils import run_bass_kernel_spmd
from concourse.vector_clock import ScopedClock

B, T, E, H, ENC, V = 64, 49, 512, 512, 1024, 32000
N_CORES = 8
VC = V // N_CORES          # 4000 vocab columns per core
BT = B * T                 # 3136
BT_PAD = 3200              # 25 chunks of 128
M_CHUNKS = BT_PAD // 128   # 25
N_TILE = 500               # 4000 = 8 * 500, fits one PSUM bank (2000 B)
N_CHUNKS = VC // N_TILE    # 8
K_CHUNKS = H // 128        # 4

LAST_RESULTS = None  # stashed BassKernelResults for test harness inspection
LAST_DEVICE_SECS = None  # wall time of the run_bass_kernel_spmd call


def _patch_tile_drain():
    """This walrus build rejects >1 sync-wait command on a CTRL/Drain
    instruction, but TileContext's tail drain carries one wait per used
    proc. Re-emit: bare drain + one SP nop per wait."""
    if getattr(tile.TileContext, "_drain_patched", False):
        return

    def _drain_and_barrier(self, tick_clock, wait_clock):
        drain_inst = self.nc.sync.drain()
        wait_clock.add_sem_waits(
            drain_inst.ins, ScopedClock({None: tick_clock.global_clock})
        )
        mi = drain_inst.ins
        si = mi.sync_info
        waits = list(si.on_wait) if si and si.on_wait else []
        if len(waits) > 1:
            si.on_wait = []
            for w in waits:
                nop = self.nc.sync.nop(nofuse=True)
                nsi = nop.ins.sync_info
                if nsi is None:
                    nop.ins.sync_info = mybir.SyncInfo(on_wait=[w], on_update=[])
                else:
                    nsi.on_wait = [w]
        self.nc.all_engine_barrier()
        assert self.sems is not None
        popped = self.nc._tile_sem_poison_stack.pop()
        assert popped is self._sem_poison
        self.nc.clear_and_free_semaphores(list(self.sems.allocated().values()))
        self.nc.all_engine_barrier()

    tile.TileContext._drain_and_barrier = _drain_and_barrier
    tile.TileContext._drain_patched = True


def _split_multi_waits(nc):
    """This walrus build rejects instructions carrying more than one sync-wait
    command. Move every wait beyond the first onto its own same-engine NoOp
    inserted immediately before the instruction."""
    uid = 0
    for f in nc.m.functions:
        for bb in f.blocks:
            new_insts = []
            changed = False
            for inst in bb.instructions:
                si = getattr(inst, "sync_info", None)
                waits = list(si.on_wait) if si and si.on_wait else []
                if len(waits) > 1:
                    changed = True
                    for w in waits[:-1]:
                        nop = mybir.InstNoOp(
                            name=f"splitw_{uid}", ins=[], outs=[]
                        )
                        uid += 1
                        nop.engine = inst.engine
                        nop.sync_info = mybir.SyncInfo(on_wait=[w], on_update=[])
                        new_insts.append(nop)
                    si.on_wait = [waits[-1]]
                new_insts.append(inst)
            if changed:
                bb.instructions = new_insts


def _sigmoid(x):
    out = np.empty_like(x)
    pos = x >= 0
    out[pos] = 1.0 / (1.0 + np.exp(-x[pos]))
    ex = np.exp(x[~pos])
    out[~pos] = ex / (1.0 + ex)
    return out


def _host_recurrence(embeddings, encoder_out, caption_lengths,
                     W_ih, W_hh, b_ih, b_hh,
                     W_init_h, b_init_h, W_init_c, b_init_c):
    """Run the (tiny, inherently sequential) LSTM recurrence; return masked
    hidden states HS[B*T, H] (row b*T+t) and decode_lengths."""
    f32 = np.float32
    h = (encoder_out @ W_init_h.T + b_init_h).astype(f32)
    c = (encoder_out @ W_init_c.T + b_init_c).astype(f32)
    decode_lengths = caption_lengths - 1
    # input-to-hidden part for all steps at once: [B, T, 4H]
    xg = (embeddings.reshape(B * T, E) @ W_ih.T + (b_ih + b_hh)).reshape(B, T, 4 * H)
    hs = np.zeros((B, T, H), f32)
    dl = np.asarray(decode_lengths)
    W_hhT = np.ascontiguousarray(W_hh.T)
    for t in range(T):
        g = xg[:, t, :] + h @ W_hhT
        i = _sigmoid(g[:, :H])
        f = _sigmoid(g[:, H:2 * H])
        gg = np.tanh(g[:, 2 * H:3 * H])
        o = _sigmoid(g[:, 3 * H:])
        c_new = f * c + i * gg
        h_new = o * np.tanh(c_new)
        m = (t < dl)[:, None]
        h = np.where(m, h_new, h)
        c = np.where(m, c_new, c)
        hs[:, t, :] = np.where(m, h_new, 0.0)
    return hs.reshape(B * T, H), decode_lengths


def _build_nc():
    _patch_tile_drain()
    nc = bass.Bass("TRN2", target_bir_lowering=False, debug=False,
                   num_devices=N_CORES)
    f32 = mybir.dt.float32
    f32r = mybir.dt.float32r
    hst = nc.dram_tensor("hst", [H, BT_PAD], f32r, kind="ExternalInput").ap()
    wfct = nc.dram_tensor("wfct", [H, VC], f32r, kind="ExternalInput").ap()
    preds = nc.dram_tensor("preds", [BT, VC], f32, kind="ExternalOutput").ap()

    with tile.TileContext(nc) as tc:
        with (
            tc.tile_pool(name="wpool", bufs=1) as wpool,
            tc.tile_pool(name="hpool", bufs=1) as hpool,
            tc.tile_pool(name="opool", bufs=6) as opool,
            tc.tile_pool(name="ppool", bufs=6, space="PSUM") as ppool,
        ):
            # resident inputs: hsT (6.5 MB) and W_fcT slice (8 MB)
            h_tiles = []
            w_tiles = []
            for k in range(K_CHUNKS):
                ht = hpool.tile([128, BT_PAD], f32r, tag=f"h{k}")
                nc.sync.dma_start(ht[:], hst[k * 128:(k + 1) * 128, :])
                h_tiles.append(ht)
                wt = wpool.tile([128, VC], f32r, tag=f"w{k}")
                nc.sync.dma_start(wt[:], wfct[k * 128:(k + 1) * 128, :])
                w_tiles.append(wt)

            for m in range(M_CHUNKS):
                rows = 128 if m < M_CHUNKS - 1 else BT - 128 * (M_CHUNKS - 1)
                ms = bass.ts(m, 128)
                for n in range(N_CHUNKS):
                    ns = bass.ts(n, N_TILE)
                    ps = ppool.tile([128, N_TILE], f32)
                    for k in range(K_CHUNKS):
                        nc.tensor.matmul(
                            ps[:], h_tiles[k][:, ms], w_tiles[k][:, ns],
                            start=(k == 0), stop=(k == K_CHUNKS - 1),
                        )
                    ot = opool.tile([128, N_TILE], f32)
                    # alternate PSUM-evict between DVE and ACT to halve the
                    # copy-engine serial time
                    if (m * N_CHUNKS + n) % 2 == 0:
                        nc.vector.tensor_copy(ot[:], ps[:])
                    else:
                        nc.scalar.copy(ot[:], ps[:])
                    nc.sync.dma_start(
                        preds[m * 128:m * 128 + rows, ns], ot[:rows, :]
                    )
    _split_multi_waits(nc)
    return nc


def kernel(embeddings, encoder_out, caption_lengths,
           W_ih, W_hh, b_ih, b_hh,
           W_init_h, b_init_h, W_init_c, b_init_c,
           W_fc, b_fc):
    global LAST_RESULTS
    embeddings = np.asarray(embeddings, np.float32)
    encoder_out = np.asarray(encoder_out, np.float32)
    caption_lengths = np.asarray(caption_lengths)

    hs, decode_lengths = _host_recurrence(
        np.asarray(embeddings), np.asarray(encoder_out), caption_lengths,
        np.asarray(W_ih, np.float32), np.asarray(W_hh, np.float32),
        np.asarray(b_ih, np.float32), np.asarray(b_hh, np.float32),
        np.asarray(W_init_h, np.float32), np.asarray(b_init_h, np.float32),
        np.asarray(W_init_c, np.float32), np.asarray(b_init_c, np.float32),
    )

    # hsT padded to [H, 3200]
    hst = np.zeros((H, BT_PAD), np.float32)
    hst[:, :BT] = hs.T
    wfct_full = np.ascontiguousarray(np.asarray(W_fc, np.float32).T)  # [H, V]

    nc = _build_nc()
    in_maps = [
        {
            "hst": hst,
            "wfct": np.ascontiguousarray(wfct_full[:, c * VC:(c + 1) * VC]),
        }
        for c in range(N_CORES)
    ]
    import time as _time
    _t0 = _time.time()
    res = run_bass_kernel_spmd(nc, in_maps, core_ids=list(range(N_CORES)))
    globals()["LAST_DEVICE_SECS"] = _time.time() - _t0
    LAST_RESULTS = res

    predictions = np.concatenate(
        [res.results[c]["preds"] for c in range(N_CORES)], axis=1
    ).reshape(B, T, V)

    b_fc = np.asarray(b_fc, np.float32)
    if b_fc.any():  # zero in this module; generic fallback
        mask = (np.arange(T)[None, :] < np.asarray(decode_lengths)[:, None])
        predictions = predictions + np.where(mask[:, :, None], b_fc, 0.0)

    return predictions, decode_lengths


# revision 6
# speedup vs baseline: 1.0254x; 1.0254x over previous
"""Trainium2 Bass kernel for nn_DecoderWithoutAttention.

Strategy (memory-regime target):
  - The module = LSTM decoder + vocab projection. The vocab projection
    (h @ W_fc.T, V=32000) dominates both FLOPs (103 GFLOP) and memory
    traffic (the 401 MB [B,T,V] output). The LSTM recurrence is tiny
    (13 GFLOP, sequential) and runs on host to produce the masked hidden
    states HS[B*T, H]; zero rows for inactive (t,b) make their
    predictions exactly 0 (b_fc == 0 in this module).
  - Device: tensor-parallel shard of the projection over vocab across 8
    cores: PRED_c[3200pad, 4000] = HS[3200, 512] @ W_fcT[:, c*4000:...].
    float32r matmuls (full PE rate at free dim >= 256), fp32 PSUM
    accumulate over K=512, DMA out 50 MB/core.
"""

import numpy as np

import concourse.bass as bass
import concourse.mybir as mybir
import concourse.tile as tile
from concourse.bass_utils import run_bass_kernel_spmd
from concourse.vector_clock import ScopedClock

B, T, E, H, ENC, V = 64, 49, 512, 512, 1024, 32000
N_CORES = 8
VC = V // N_CORES          # 4000 vocab columns per core
BT = B * T                 # 3136
BT_PAD = 3200              # 25 chunks of 128
M_CHUNKS = BT_PAD // 128   # 25
N_TILE = 500               # 4000 = 8 * 500, fits one PSUM bank (2000 B)
N_CHUNKS = VC // N_TILE    # 8
K_CHUNKS = H // 128        # 4

LAST_RESULTS = None  # stashed BassKernelResults for test harness inspection
LAST_DEVICE_SECS = None  # wall time of the run_bass_kernel_spmd call


def _patch_tile_drain():
    """This walrus build rejects >1 sync-wait command on a CTRL/Drain
    instruction, but TileContext's tail drain carries one wait per used
    proc. Re-emit: bare drain + one SP nop per wait."""
    if getattr(tile.TileContext, "_drain_patched", False):
        return

    def _drain_and_barrier(self, tick_clock, wait_clock):
        drain_inst = self.nc.sync.drain()
        wait_clock.add_sem_waits(
            drain_inst.ins, ScopedClock({None: tick_clock.global_clock})
        )
        mi = drain_inst.ins
        si = mi.sync_info
        waits = list(si.on_wait) if si and si.on_wait else []
        if len(waits) > 1:
            si.on_wait = []
            for w in waits:
                nop = self.nc.sync.nop(nofuse=True)
                nsi = nop.ins.sync_info
                if nsi is None:
                    nop.ins.sync_info = mybir.SyncInfo(on_wait=[w], on_update=[])
                else:
                    nsi.on_wait = [w]
        self.nc.all_engine_barrier()
        assert self.sems is not None
        popped = self.nc._tile_sem_poison_stack.pop()
        assert popped is self._sem_poison
        self.nc.clear_and_free_semaphores(list(self.sems.allocated().values()))
        self.nc.all_engine_barrier()

    tile.TileContext._drain_and_barrier = _drain_and_barrier
    tile.TileContext._drain_patched = True


def _split_multi_waits(nc):
    """This walrus build rejects instructions carrying more than one sync-wait
    command. Move every wait beyond the first onto its own same-engine NoOp
    inserted immediately before the instruction."""
    uid = 0
    for f in nc.m.functions:
        for bb in f.blocks:
            new_insts = []
            changed = False
            for inst in bb.instructions:
                si = getattr(inst, "sync_info", None)
                waits = list(si.on_wait) if si and si.on_wait else []
                if len(waits) > 1:
                    changed = True
                    for w in waits[:-1]:
                        nop = mybir.InstNoOp(
                            name=f"splitw_{uid}", ins=[], outs=[]
                        )
                        uid += 1
                        nop.engine = inst.engine
                        nop.sync_info = mybir.SyncInfo(on_wait=[w], on_update=[])
                        new_insts.append(nop)
                    si.on_wait = [waits[-1]]
                new_insts.append(inst)
            if changed:
                bb.instructions = new_insts


def _sigmoid(x):
    out = np.empty_like(x)
    pos = x >= 0
    out[pos] = 1.0 / (1.0 + np.exp(-x[pos]))
    ex = np.exp(x[~pos])
    out[~pos] = ex / (1.0 + ex)
    return out


def _host_recurrence(embeddings, encoder_out, caption_lengths,
                     W_ih, W_hh, b_ih, b_hh,
                     W_init_h, b_init_h, W_init_c, b_init_c):
    """Run the (tiny, inherently sequential) LSTM recurrence; return masked
    hidden states HS[B*T, H] (row b*T+t) and decode_lengths."""
    f32 = np.float32
    h = (encoder_out @ W_init_h.T + b_init_h).astype(f32)
    c = (encoder_out @ W_init_c.T + b_init_c).astype(f32)
    decode_lengths = caption_lengths - 1
    # input-to-hidden part for all steps at once: [B, T, 4H]
    xg = (embeddings.reshape(B * T, E) @ W_ih.T + (b_ih + b_hh)).reshape(B, T, 4 * H)
    hs = np.zeros((B, T, H), f32)
    dl = np.asarray(decode_lengths)
    W_hhT = np.ascontiguousarray(W_hh.T)
    for t in range(T):
        g = xg[:, t, :] + h @ W_hhT
        i = _sigmoid(g[:, :H])
        f = _sigmoid(g[:, H:2 * H])
        gg = np.tanh(g[:, 2 * H:3 * H])
        o = _sigmoid(g[:, 3 * H:])
        c_new = f * c + i * gg
        h_new = o * np.tanh(c_new)
        m = (t < dl)[:, None]
        h = np.where(m, h_new, h)
        c = np.where(m, c_new, c)
        hs[:, t, :] = np.where(m, h_new, 0.0)
    return hs.reshape(B * T, H), decode_lengths


def _build_nc():
    _patch_tile_drain()
    nc = bass.Bass("TRN2", target_bir_lowering=False, debug=False,
                   num_devices=N_CORES)
    f32 = mybir.dt.float32
    f32r = mybir.dt.float32r
    hst = nc.dram_tensor("hst", [H, BT_PAD], f32r, kind="ExternalInput").ap()
    wfct = nc.dram_tensor("wfct", [H, VC], f32r, kind="ExternalInput").ap()
    preds = nc.dram_tensor("preds", [BT, VC], f32, kind="ExternalOutput").ap()

    with tile.TileContext(nc) as tc:
        with (
            tc.tile_pool(name="wpool", bufs=1) as wpool,
            tc.tile_pool(name="hpool", bufs=1) as hpool,
            tc.tile_pool(name="opool", bufs=8) as opool,
            tc.tile_pool(name="ppool", bufs=8, space="PSUM") as ppool,
        ):
            # resident inputs: hsT (6.5 MB) and W_fcT slice (8 MB)
            h_tiles = []
            w_tiles = []
            for k in range(K_CHUNKS):
                ht = hpool.tile([128, BT_PAD], f32r, tag=f"h{k}")
                nc.sync.dma_start(ht[:], hst[k * 128:(k + 1) * 128, :])
                h_tiles.append(ht)
                wt = wpool.tile([128, VC], f32r, tag=f"w{k}")
                nc.sync.dma_start(wt[:], wfct[k * 128:(k + 1) * 128, :])
                w_tiles.append(wt)

            for m in range(M_CHUNKS):
                rows = 128 if m < M_CHUNKS - 1 else BT - 128 * (M_CHUNKS - 1)
                ms = bass.ts(m, 128)
                for n in range(N_CHUNKS):
                    ns = bass.ts(n, N_TILE)
                    ps = ppool.tile([128, N_TILE], f32)
                    for k in range(K_CHUNKS):
                        nc.tensor.matmul(
                            ps[:], h_tiles[k][:, ms], w_tiles[k][:, ns],
                            start=(k == 0), stop=(k == K_CHUNKS - 1),
                        )
                    ot = opool.tile([128, N_TILE], f32)
                    # alternate PSUM-evict between DVE and ACT to halve the
                    # copy-engine serial time
                    if (m * N_CHUNKS + n) % 2 == 0:
                        nc.vector.tensor_copy(ot[:], ps[:])
                    else:
                        nc.scalar.copy(ot[:], ps[:])
                    nc.sync.dma_start(
                        preds[m * 128:m * 128 + rows, ns], ot[:rows, :]
                    )
    _split_multi_waits(nc)
    return nc


def kernel(embeddings, encoder_out, caption_lengths,
           W_ih, W_hh, b_ih, b_hh,
           W_init_h, b_init_h, W_init_c, b_init_c,
           W_fc, b_fc):
    global LAST_RESULTS
    embeddings = np.asarray(embeddings, np.float32)
    encoder_out = np.asarray(encoder_out, np.float32)
    caption_lengths = np.asarray(caption_lengths)

    hs, decode_lengths = _host_recurrence(
        np.asarray(embeddings), np.asarray(encoder_out), caption_lengths,
        np.asarray(W_ih, np.float32), np.asarray(W_hh, np.float32),
        np.asarray(b_ih, np.float32), np.asarray(b_hh, np.float32),
        np.asarray(W_init_h, np.float32), np.asarray(b_init_h, np.float32),
        np.asarray(W_init_c, np.float32), np.asarray(b_init_c, np.float32),
    )

    # hsT padded to [H, 3200]
    hst = np.zeros((H, BT_PAD), np.float32)
    hst[:, :BT] = hs.T
    wfct_full = np.ascontiguousarray(np.asarray(W_fc, np.float32).T)  # [H, V]

    nc = _build_nc()
    in_maps = [
        {
            "hst": hst,
            "wfct": np.ascontiguousarray(wfct_full[:, c * VC:(c + 1) * VC]),
        }
        for c in range(N_CORES)
    ]
    import time as _time
    _t0 = _time.time()
    res = run_bass_kernel_spmd(nc, in_maps, core_ids=list(range(N_CORES)))
    globals()["LAST_DEVICE_SECS"] = _time.time() - _t0
    LAST_RESULTS = res

    predictions = np.concatenate(
        [res.results[c]["preds"] for c in range(N_CORES)], axis=1
    ).reshape(B, T, V)

    b_fc = np.asarray(b_fc, np.float32)
    if b_fc.any():  # zero in this module; generic fallback
        mask = (np.arange(T)[None, :] < np.asarray(decode_lengths)[:, None])
        predictions = predictions + np.where(mask[:, :, None], b_fc, 0.0)

    return predictions, decode_lengths


# revision 8
# speedup vs baseline: 1.5672x; 1.5284x over previous
"""Trainium2 Bass kernel for nn_DecoderWithoutAttention.

Strategy (memory-regime target):
  - The module = LSTM decoder + vocab projection. The vocab projection
    (h @ W_fc.T, V=32000) dominates both FLOPs (103 GFLOP) and memory
    traffic (the 401 MB [B,T,V] output). The LSTM recurrence is tiny
    (13 GFLOP, sequential) and runs on host to produce the masked hidden
    states HS[B*T, H]; zero rows for inactive (t,b) make their
    predictions exactly 0 (b_fc == 0 in this module).
  - Device: tensor-parallel shard of the projection over vocab across 8
    cores: PRED_c[3200pad, 4000] = HS[3200, 512] @ W_fcT[:, c*4000:...].
    float32r matmuls (full PE rate at free dim >= 256), fp32 PSUM
    accumulate over K=512, DMA out 50 MB/core.
"""

import numpy as np

import concourse.bass as bass
import concourse.mybir as mybir
import concourse.tile as tile
from concourse.bass_utils import run_bass_kernel_spmd
from concourse.vector_clock import ScopedClock

B, T, E, H, ENC, V = 64, 49, 512, 512, 1024, 32000
N_CORES = 8
VC = V // N_CORES          # 4000 vocab columns per core
BT = B * T                 # 3136
BT_PAD = 3200              # 25 chunks of 128
M_CHUNKS = BT_PAD // 128   # 25
N_TILE = 500               # 4000 = 8 * 500, fits one PSUM bank (2000 B)
N_CHUNKS = VC // N_TILE    # 8
K_CHUNKS = H // 128        # 4

LAST_RESULTS = None  # stashed BassKernelResults for test harness inspection
LAST_DEVICE_SECS = None  # wall time of the run_bass_kernel_spmd call


def _patch_tile_drain():
    """This walrus build rejects >1 sync-wait command on a CTRL/Drain
    instruction, but TileContext's tail drain carries one wait per used
    proc. Re-emit: bare drain + one SP nop per wait."""
    if getattr(tile.TileContext, "_drain_patched", False):
        return

    def _drain_and_barrier(self, tick_clock, wait_clock):
        drain_inst = self.nc.sync.drain()
        wait_clock.add_sem_waits(
            drain_inst.ins, ScopedClock({None: tick_clock.global_clock})
        )
        mi = drain_inst.ins
        si = mi.sync_info
        waits = list(si.on_wait) if si and si.on_wait else []
        if len(waits) > 1:
            si.on_wait = []
            for w in waits:
                nop = self.nc.sync.nop(nofuse=True)
                nsi = nop.ins.sync_info
                if nsi is None:
                    nop.ins.sync_info = mybir.SyncInfo(on_wait=[w], on_update=[])
                else:
                    nsi.on_wait = [w]
        self.nc.all_engine_barrier()
        assert self.sems is not None
        popped = self.nc._tile_sem_poison_stack.pop()
        assert popped is self._sem_poison
        self.nc.clear_and_free_semaphores(list(self.sems.allocated().values()))
        self.nc.all_engine_barrier()

    tile.TileContext._drain_and_barrier = _drain_and_barrier
    tile.TileContext._drain_patched = True


def _split_multi_waits(nc):
    """This walrus build rejects instructions carrying more than one sync-wait
    command. Move every wait beyond the first onto its own same-engine NoOp
    inserted immediately before the instruction."""
    uid = 0
    for f in nc.m.functions:
        for bb in f.blocks:
            new_insts = []
            changed = False
            for inst in bb.instructions:
                si = getattr(inst, "sync_info", None)
                waits = list(si.on_wait) if si and si.on_wait else []
                if len(waits) > 1:
                    changed = True
                    for w in waits[:-1]:
                        nop = mybir.InstNoOp(
                            name=f"splitw_{uid}", ins=[], outs=[]
                        )
                        uid += 1
                        nop.engine = inst.engine
                        nop.sync_info = mybir.SyncInfo(on_wait=[w], on_update=[])
                        new_insts.append(nop)
                    si.on_wait = [waits[-1]]
                new_insts.append(inst)
            if changed:
                bb.instructions = new_insts


def _sigmoid(x):
    out = np.empty_like(x)
    pos = x >= 0
    out[pos] = 1.0 / (1.0 + np.exp(-x[pos]))
    ex = np.exp(x[~pos])
    out[~pos] = ex / (1.0 + ex)
    return out


def _host_recurrence(embeddings, encoder_out, caption_lengths,
                     W_ih, W_hh, b_ih, b_hh,
                     W_init_h, b_init_h, W_init_c, b_init_c):
    """Run the (tiny, inherently sequential) LSTM recurrence; return masked
    hidden states HS[B*T, H] (row b*T+t) and decode_lengths."""
    f32 = np.float32
    h = (encoder_out @ W_init_h.T + b_init_h).astype(f32)
    c = (encoder_out @ W_init_c.T + b_init_c).astype(f32)
    decode_lengths = caption_lengths - 1
    # input-to-hidden part for all steps at once: [B, T, 4H]
    xg = (embeddings.reshape(B * T, E) @ W_ih.T + (b_ih + b_hh)).reshape(B, T, 4 * H)
    hs = np.zeros((B, T, H), f32)
    dl = np.asarray(decode_lengths)
    W_hhT = np.ascontiguousarray(W_hh.T)
    for t in range(T):
        g = xg[:, t, :] + h @ W_hhT
        i = _sigmoid(g[:, :H])
        f = _sigmoid(g[:, H:2 * H])
        gg = np.tanh(g[:, 2 * H:3 * H])
        o = _sigmoid(g[:, 3 * H:])
        c_new = f * c + i * gg
        h_new = o * np.tanh(c_new)
        m = (t < dl)[:, None]
        h = np.where(m, h_new, h)
        c = np.where(m, c_new, c)
        hs[:, t, :] = np.where(m, h_new, 0.0)
    return hs.reshape(B * T, H), decode_lengths


def _build_nc():
    _patch_tile_drain()
    nc = bass.Bass("TRN2", target_bir_lowering=False, debug=False,
                   num_devices=N_CORES)
    f32 = mybir.dt.float32
    f32r = mybir.dt.float32r
    hst = nc.dram_tensor("hst", [H, BT_PAD], f32r, kind="ExternalInput").ap()
    wfct = nc.dram_tensor("wfct", [H, VC], f32r, kind="ExternalInput").ap()
    preds = nc.dram_tensor("preds", [BT, VC], f32, kind="ExternalOutput").ap()

    with tile.TileContext(nc) as tc:
        with (
            tc.tile_pool(name="wpool", bufs=1) as wpool,
            tc.tile_pool(name="hpool", bufs=1) as hpool,
            tc.tile_pool(name="opool", bufs=8) as opool,
            tc.tile_pool(name="ppool", bufs=8, space="PSUM") as ppool,
        ):
            # resident inputs: hsT (6.5 MB) and W_fcT slice (8 MB), loaded in
            # consumption order so the first matmul group only waits on
            # ~2.6 MB (hsT col-block 0 + W n-chunk 0), not all 14.7 MB
            CB = 640                       # hsT col-block: 5 m-chunks each
            N_CB = BT_PAD // CB            # 5
            h_tiles = {}
            w_tiles = {}

            def load_h(k, cb):
                ht = hpool.tile([128, CB], f32r, tag=f"h{k}_{cb}")
                nc.sync.dma_start(
                    ht[:], hst[k * 128:(k + 1) * 128, cb * CB:(cb + 1) * CB]
                )
                h_tiles[(k, cb)] = ht

            def load_w(k, n):
                wt = wpool.tile([128, N_TILE], f32r, tag=f"w{k}_{n}")
                nc.sync.dma_start(
                    wt[:], wfct[k * 128:(k + 1) * 128,
                                n * N_TILE:(n + 1) * N_TILE]
                )
                w_tiles[(k, n)] = wt

            for k in range(K_CHUNKS):
                load_h(k, 0)
            for k in range(K_CHUNKS):
                load_w(k, 0)
            for cb in range(1, N_CB):
                for k in range(K_CHUNKS):
                    load_h(k, cb)
            for n in range(1, N_CHUNKS):
                for k in range(K_CHUNKS):
                    load_w(k, n)

            for m in range(M_CHUNKS):
                rows = 128 if m < M_CHUNKS - 1 else BT - 128 * (M_CHUNKS - 1)
                cb, mo = divmod(m, CB // 128)
                ms = bass.ts(mo, 128)
                for n in range(N_CHUNKS):
                    ns = bass.ts(n, N_TILE)
                    ps = ppool.tile([128, N_TILE], f32)
                    for k in range(K_CHUNKS):
                        nc.tensor.matmul(
                            ps[:], h_tiles[(k, cb)][:, ms], w_tiles[(k, n)][:],
                            start=(k == 0), stop=(k == K_CHUNKS - 1),
                        )
                    ot = opool.tile([128, N_TILE], f32)
                    # alternate PSUM-evict between DVE and ACT to halve the
                    # copy-engine serial time
                    if (m * N_CHUNKS + n) % 2 == 0:
                        nc.vector.tensor_copy(ot[:], ps[:])
                    else:
                        nc.scalar.copy(ot[:], ps[:])
                    nc.sync.dma_start(
                        preds[m * 128:m * 128 + rows, ns], ot[:rows, :]
                    )
    _split_multi_waits(nc)
    return nc


def kernel(embeddings, encoder_out, caption_lengths,
           W_ih, W_hh, b_ih, b_hh,
           W_init_h, b_init_h, W_init_c, b_init_c,
           W_fc, b_fc):
    global LAST_RESULTS
    embeddings = np.asarray(embeddings, np.float32)
    encoder_out = np.asarray(encoder_out, np.float32)
    caption_lengths = np.asarray(caption_lengths)

    hs, decode_lengths = _host_recurrence(
        np.asarray(embeddings), np.asarray(encoder_out), caption_lengths,
        np.asarray(W_ih, np.float32), np.asarray(W_hh, np.float32),
        np.asarray(b_ih, np.float32), np.asarray(b_hh, np.float32),
        np.asarray(W_init_h, np.float32), np.asarray(b_init_h, np.float32),
        np.asarray(W_init_c, np.float32), np.asarray(b_init_c, np.float32),
    )

    # hsT padded to [H, 3200]
    hst = np.zeros((H, BT_PAD), np.float32)
    hst[:, :BT] = hs.T
    wfct_full = np.ascontiguousarray(np.asarray(W_fc, np.float32).T)  # [H, V]

    nc = _build_nc()
    in_maps = [
        {
            "hst": hst,
            "wfct": np.ascontiguousarray(wfct_full[:, c * VC:(c + 1) * VC]),
        }
        for c in range(N_CORES)
    ]
    import time as _time
    _t0 = _time.time()
    res = run_bass_kernel_spmd(nc, in_maps, core_ids=list(range(N_CORES)))
    globals()["LAST_DEVICE_SECS"] = _time.time() - _t0
    LAST_RESULTS = res

    predictions = np.concatenate(
        [res.results[c]["preds"] for c in range(N_CORES)], axis=1
    ).reshape(B, T, V)

    b_fc = np.asarray(b_fc, np.float32)
    if b_fc.any():  # zero in this module; generic fallback
        mask = (np.arange(T)[None, :] < np.asarray(decode_lengths)[:, None])
        predictions = predictions + np.where(mask[:, :, None], b_fc, 0.0)

    return predictions, decode_lengths
